# revision 1
# baseline (speedup 1.0000x reference)
"""DeepseekV3 MoE layer on 8 Trainium2 NeuronCores (expert-parallel).

Contract: kernel(**inputs) takes the FULL unsharded inputs and returns the
FULL output [4, 2048, 2048] f32.

Strategy:
  - Routing (sigmoid gate + group-limited top-6) computed on host in numpy.
  - Expert parallelism: 32 experts -> 8 cores x 4 slots. Experts are
    assigned to (core, slot) by sorted token count so every core runs an
    identical static program (slot capacities shared across cores).
  - Token dispatch on host: tokens gathered/padded per expert, transposed
    to feature-major [H, ncap] blocks; per-token routing weights applied
    on device; host scatter-adds the (disjoint) results.
  - Shared MLP data-parallel over tokens (1024 tokens per core).
  - Device kernel: fp32r matmuls (full PE rate, ~1.5e-4 matmul error),
    silu/mul on ACT/DVE, weights streamed from HBM once per token chunk.
"""
import sys
import os

sys.path.insert(0, "/opt/trn_rl_repo")

import numpy as np

import concourse.bacc as bacc_mod
import concourse.mybir as mybir
import concourse.tile as tile
from concourse.bass_utils import run_bass_kernel_spmd

F32 = mybir.dt.float32
F32R = mybir.dt.float32r
P = 128

# Problem constants (hardcoded per contract)
B, S, H = 4, 2048, 2048
T = B * S                      # 8192 tokens
E = 32                         # experts
TOPK = 6
N_GROUPS = 4
N_LIMITED = 2
MI = 1408                      # expert intermediate
SH = 2816                      # shared intermediate
NH = H // P                    # 16 h-tiles
NM = MI // P                   # 11 m-tiles (expert)
NMS = SH // P                  # 22 m-tiles (shared)
NCORES = 8
NSLOTS = 4
HC = 512                       # stage-2 output column chunk
NHC = H // HC                  # 4
TOK_SH = T // NCORES           # 1024 shared-MLP tokens per core


def _round_up(x, m):
    return ((x + m - 1) // m) * m


_GOOD_CHUNKS = (896, 768, 512, 384, 256)  # ck % 512 in {0, 256, 384}: no <256 pieces


def _chunks_of(cap):
    """Split a slot capacity (multiple of 128) into the fewest chunks from
    _GOOD_CHUNKS (fp32r needs moving pieces >=256 for full PE rate)."""
    best = {0: []}
    for c in range(128, cap + 1, 128):
        cands = []
        for g in _GOOD_CHUNKS:
            if g <= c and (c - g) in best:
                cands.append([g] + best[c - g])
        if c in (128,) or not cands:
            if (c - 128) in best:
                cands.append([128] + best[c - 128])   # last-resort tiny chunk
        if cands:
            best[c] = min(cands, key=lambda l: (len(l), l.count(128)))
    return sorted(best[cap], reverse=True)


def _pieces_of(ck):
    """Split a chunk into moving-dim pieces of <=512 (bank-aligned)."""
    out = []
    off = 0
    while off < ck:
        pl = min(512, ck - off)
        out.append((off, pl))
        off += pl
    return out


def _mlp_chunk(nc, wpool, iopool, respool, pspool, xt_dram, toff, ck,
               wg_dram, wu_dram, wd_dram, n_mt, rws, rw_base, y_dram):
    """One token chunk through gate/up/silu*up/down (+ routing-weight scale).

    xt_dram: [H, ncap-like] feature-major tokens (f32r dram)
    wg_dram/wu_dram: [n_mt, 128, H]   (m-tile, p, ko*128)
    wd_dram: [NHC, 128, n_mt*512]     (hc, p(m-row), mt*512)
    rws: resident [128, NT] routing-weight tile or None
    y_dram: [ntok, H] output rows
    """
    ACT = mybir.ActivationFunctionType
    # ---- load + round X^T chunk (per h-tile so PE can start early) ----
    xtr = respool.tile([P, NH, ck], F32R, tag="xtr")
    for h in range(NH):
        nc.sync.dma_start(xtr[:, h], xt_dram[h * P:(h + 1) * P, toff:toff + ck])
        nc.vector.tensor_copy(out=xtr[:, h], in_=xtr[:, h])

    A = respool.tile([P, n_mt, ck], F32R, tag="A")
    pieces = _pieces_of(ck)
    # ---- stage 1: G = X@Wg, U = X@Wu, A = silu(G)*U  (feature-major) ----
    for m in range(n_mt):
        wgr = wpool.tile([P, NH, P], F32R, tag="wgr")
        wur = wpool.tile([P, NH, P], F32R, tag="wur")
        nc.sync.dma_start(wgr[:], wg_dram[m].rearrange("p (ko x) -> p ko x", x=P))
        nc.sync.dma_start(wur[:], wu_dram[m].rearrange("p (ko x) -> p ko x", x=P))
        nc.vector.tensor_copy(out=wgr[:], in_=wgr[:])
        nc.vector.tensor_copy(out=wur[:], in_=wur[:])
        for (po, pl) in pieces:
            g = pspool.tile([P, pl], F32, tag="g")
            u = pspool.tile([P, pl], F32, tag="u")
            for h in range(NH):
                nc.tensor.matmul(g[:], wgr[:, h], xtr[:, h, po:po + pl],
                                 start=(h == 0), stop=(h == NH - 1))
            for h in range(NH):
                nc.tensor.matmul(u[:], wur[:, h], xtr[:, h, po:po + pl],
                                 start=(h == 0), stop=(h == NH - 1))
            sg = iopool.tile([P, pl], F32, tag="sg")
            nc.scalar.activation(out=sg[:], in_=g[:], func=ACT.Silu)
            # fp32r output = the rounding producer for stage 2
            nc.vector.tensor_mul(out=A[:, m, po:po + pl], in0=sg[:], in1=u[:])

    # ---- stage 2: Y = A @ Wd, scaled by routing weight, token-major ----
    ntt = ck // P
    for hc in range(NHC):
        wdr = wpool.tile([P, n_mt, HC], F32R, tag="wdr")
        nc.sync.dma_start(wdr[:], wd_dram[hc].rearrange("p (mt x) -> p mt x", x=HC))
        nc.vector.tensor_copy(out=wdr[:], in_=wdr[:])
        for t in range(ntt):
            yp = pspool.tile([P, HC], F32, tag="y")
            for m in range(n_mt):
                nc.tensor.matmul(yp[:], A[:, m, t * P:(t + 1) * P], wdr[:, m],
                                 start=(m == 0), stop=(m == n_mt - 1))
            ysb = iopool.tile([P, HC], F32, tag="ysb")
            if rws is None:
                nc.scalar.copy(ysb[:], yp[:])
            else:
                ti = rw_base + t
                nc.scalar.mul(ysb[:], yp[:], rws[:, ti:ti + 1])
            nc.sync.dma_start(
                y_dram[toff + t * P: toff + (t + 1) * P, hc * HC:(hc + 1) * HC],
                ysb[:])


def build_bass(slot_caps):
    ncap = sum(slot_caps)
    nt = ncap // P
    nc = bacc_mod.Bacc(trn_type="TRN2")

    xt = nc.dram_tensor("xt", [H, ncap], F32R, kind="ExternalInput")
    rw = nc.dram_tensor("rw", [P, nt], F32, kind="ExternalInput")
    wg = nc.dram_tensor("wg", [NSLOTS, NM, P, H], F32R, kind="ExternalInput")
    wu = nc.dram_tensor("wu", [NSLOTS, NM, P, H], F32R, kind="ExternalInput")
    wd = nc.dram_tensor("wd", [NSLOTS, NHC, P, NM * HC], F32R, kind="ExternalInput")
    xts = nc.dram_tensor("xts", [H, TOK_SH], F32R, kind="ExternalInput")
    swg = nc.dram_tensor("swg", [NMS, P, H], F32R, kind="ExternalInput")
    swu = nc.dram_tensor("swu", [NMS, P, H], F32R, kind="ExternalInput")
    swd = nc.dram_tensor("swd", [NHC, P, NMS * HC], F32R, kind="ExternalInput")
    y = nc.dram_tensor("y", [ncap, H], F32, kind="ExternalOutput")
    ys = nc.dram_tensor("ys", [TOK_SH, H], F32, kind="ExternalOutput")

    with tile.TileContext(nc) as tc:
        with tc.tile_pool(name="rwp", bufs=1) as rwp:
            rws = rwp.tile([P, nt], F32)
            nc.sync.dma_start(rws[:], rw[:])

            # ---- expert phase ----
            with tc.tile_pool(name="eres", bufs=1) as eres, \
                 tc.tile_pool(name="ew", bufs=2) as ew, \
                 tc.tile_pool(name="eio", bufs=3) as eio, \
                 tc.tile_pool(name="eps", bufs=2, space="PSUM") as eps:
                soff = 0
                for s in range(NSLOTS):
                    coff = 0
                    for ck in _chunks_of(slot_caps[s]):
                        toff = soff + coff
                        _mlp_chunk(nc, ew, eio, eres, eps, xt, toff, ck,
                                   wg[s], wu[s], wd[s], NM,
                                   rws, toff // P, y)
                        coff += ck
                    soff += slot_caps[s]

            # ---- shared-expert phase: one 1024-token chunk, weights
            # streamed exactly once; xtr pool closed before the down-proj
            # slices are allocated so everything fits in SBUF ----
            ACT = mybir.ActivationFunctionType
            with tc.tile_pool(name="sres", bufs=1) as sres, \
                 tc.tile_pool(name="sio", bufs=2) as sio, \
                 tc.tile_pool(name="sps", bufs=2, space="PSUM") as sps:
                A = sres.tile([P, NMS, TOK_SH], F32R)
                with tc.tile_pool(name="sx", bufs=1) as sxp, \
                     tc.tile_pool(name="s1w", bufs=2) as s1w:
                    xtr = sxp.tile([P, NH, TOK_SH], F32R)
                    for h in range(NH):
                        nc.sync.dma_start(xtr[:, h],
                                          xts[h * P:(h + 1) * P, :])
                        nc.vector.tensor_copy(out=xtr[:, h], in_=xtr[:, h])
                    for m in range(NMS):
                        wgr = s1w.tile([P, NH, P], F32R, tag="wgr")
                        wur = s1w.tile([P, NH, P], F32R, tag="wur")
                        nc.sync.dma_start(
                            wgr[:], swg[m].rearrange("p (ko x) -> p ko x", x=P))
                        nc.sync.dma_start(
                            wur[:], swu[m].rearrange("p (ko x) -> p ko x", x=P))
                        nc.vector.tensor_copy(out=wgr[:], in_=wgr[:])
                        nc.vector.tensor_copy(out=wur[:], in_=wur[:])
                        for (po, pl) in _pieces_of(TOK_SH):
                            g = sps.tile([P, pl], F32, tag="g")
                            u = sps.tile([P, pl], F32, tag="u")
                            for h in range(NH):
                                nc.tensor.matmul(g[:], wgr[:, h],
                                                 xtr[:, h, po:po + pl],
                                                 start=(h == 0), stop=(h == NH - 1))
                            for h in range(NH):
                                nc.tensor.matmul(u[:], wur[:, h],
                                                 xtr[:, h, po:po + pl],
                                                 start=(h == 0), stop=(h == NH - 1))
                            sg = sio.tile([P, pl], F32, tag="sg")
                            nc.scalar.activation(out=sg[:], in_=g[:], func=ACT.Silu)
                            nc.vector.tensor_mul(out=A[:, m, po:po + pl],
                                                 in0=sg[:], in1=u[:])
                with tc.tile_pool(name="s2w", bufs=2) as s2w:
                    for hc in range(NHC):
                        wdr = s2w.tile([P, NMS, HC], F32R, tag="wdr")
                        nc.sync.dma_start(
                            wdr[:], swd[hc].rearrange("p (mt x) -> p mt x", x=HC))
                        nc.vector.tensor_copy(out=wdr[:], in_=wdr[:])
                        for t in range(TOK_SH // P):
                            yp = sps.tile([P, HC], F32, tag="y")
                            for m in range(NMS):
                                nc.tensor.matmul(yp[:], A[:, m, t * P:(t + 1) * P],
                                                 wdr[:, m],
                                                 start=(m == 0), stop=(m == NMS - 1))
                            ysb = sio.tile([P, HC], F32, tag="ysb")
                            nc.scalar.copy(ysb[:], yp[:])
                            nc.sync.dma_start(
                                ys[t * P:(t + 1) * P, hc * HC:(hc + 1) * HC], ysb[:])
    nc.finalize()
    return nc


def _route(x, gate_w):
    """Replicate the reference routing in numpy fp32."""
    logits = x @ gate_w                                   # [T, E]
    scores = 1.0 / (1.0 + np.exp(-logits))
    sg = scores.reshape(T, N_GROUPS, E // N_GROUPS)
    group_scores = sg.max(axis=-1)
    top_groups = np.argsort(-group_scores, axis=1, kind="stable")[:, :N_LIMITED]
    mask = np.ones((T, N_GROUPS), dtype=bool)
    mask[np.arange(T)[:, None], top_groups] = False
    sgm = np.where(mask[:, :, None], -np.inf, sg).reshape(T, E)
    sel = np.argsort(-sgm, axis=1, kind="stable")[:, :TOPK]     # [T, K]
    w = np.take_along_axis(scores, sel, axis=1)
    w = w / w.sum(axis=1, keepdims=True)
    return sel.astype(np.int64), w.astype(np.float32)


def prepare(hidden_states, gate_w, w_gate, w_up, w_down, sw_gate, sw_up, sw_down):
    """Host-side routing + sharding. Returns (slot_caps, in_maps, meta)."""
    x = np.ascontiguousarray(np.asarray(hidden_states, dtype=np.float32).reshape(T, H))
    gate_w = np.asarray(gate_w, dtype=np.float32)
    w_gate = np.asarray(w_gate, dtype=np.float32)
    w_up = np.asarray(w_up, dtype=np.float32)
    w_down = np.asarray(w_down, dtype=np.float32)
    sw_gate = np.asarray(sw_gate, dtype=np.float32)
    sw_up = np.asarray(sw_up, dtype=np.float32)
    sw_down = np.asarray(sw_down, dtype=np.float32)

    # ---- 1. routing ----
    sel, wts = _route(x, gate_w)
    sel_flat = sel.ravel()                       # pair index -> expert
    counts = np.bincount(sel_flat, minlength=E)

    # ---- 2. expert -> (core, slot) assignment ----
    order = np.argsort(-counts, kind="stable")   # experts by count desc
    slot_caps = []
    assign = np.empty((NCORES, NSLOTS), dtype=np.int64)
    for s in range(NSLOTS):
        grp = order[s * NCORES:(s + 1) * NCORES]
        assign[:, s] = grp
        slot_caps.append(max(P, _round_up(int(counts[grp].max()), P)))
    ncap = sum(slot_caps)
    soffs = np.cumsum([0] + slot_caps)[:-1]

    # token-pair rows per expert, ascending pair index (stable)
    rows_of = [np.flatnonzero(sel_flat == e) for e in range(E)]

    # ---- 3. per-core inputs ----
    # shared tensors (identical on every core)
    swg_t = np.ascontiguousarray(
        sw_gate.reshape(NH, P, NMS, P).transpose(2, 1, 0, 3).reshape(NMS, P, H))
    swu_t = np.ascontiguousarray(
        sw_up.reshape(NH, P, NMS, P).transpose(2, 1, 0, 3).reshape(NMS, P, H))
    swd_t = np.ascontiguousarray(
        sw_down.reshape(NMS, P, NHC, HC).transpose(2, 1, 0, 3).reshape(NHC, P, NMS * HC))

    in_maps = []
    for c in range(NCORES):
        el = assign[c]                            # 4 expert ids
        xt_c = np.zeros((ncap, H), dtype=np.float32)
        rw_c = np.zeros(ncap, dtype=np.float32)
        for s in range(NSLOTS):
            e = el[s]
            r = rows_of[e]
            n = len(r)
            xt_c[soffs[s]:soffs[s] + n] = x[r // TOPK]
            rw_c[soffs[s]:soffs[s] + n] = wts[r // TOPK, r % TOPK]
        xt_c = np.ascontiguousarray(xt_c.T)       # [H, ncap]
        rw_t = np.ascontiguousarray(rw_c.reshape(ncap // P, P).T)   # [128, nt]

        wg_c = np.ascontiguousarray(
            w_gate[el].reshape(NSLOTS, NH, P, NM, P)
            .transpose(0, 3, 2, 1, 4).reshape(NSLOTS, NM, P, H))
        wu_c = np.ascontiguousarray(
            w_up[el].reshape(NSLOTS, NH, P, NM, P)
            .transpose(0, 3, 2, 1, 4).reshape(NSLOTS, NM, P, H))
        wd_c = np.ascontiguousarray(
            w_down[el].reshape(NSLOTS, NM, P, NHC, HC)
            .transpose(0, 3, 2, 1, 4).reshape(NSLOTS, NHC, P, NM * HC))

        xts_c = np.ascontiguousarray(x[c * TOK_SH:(c + 1) * TOK_SH].T)  # [H, 1024]

        in_maps.append({
            "xt": xt_c, "rw": rw_t,
            "wg": wg_c, "wu": wu_c, "wd": wd_c,
            "xts": xts_c, "swg": swg_t, "swu": swu_t, "swd": swd_t,
        })

    meta = {"rows_of": rows_of, "assign": assign, "soffs": soffs}
    return slot_caps, in_maps, meta


def combine(results, meta):
    """Host-side unshard: scatter expert outputs back + add shared."""
    rows_of, assign, soffs = meta["rows_of"], meta["assign"], meta["soffs"]
    d_pairs = np.empty((T * TOPK, H), dtype=np.float32)
    for c in range(NCORES):
        y_c = results[c]["y"]
        for s in range(NSLOTS):
            r = rows_of[assign[c, s]]
            d_pairs[r] = y_c[soffs[s]:soffs[s] + len(r)]
    expert_out = d_pairs.reshape(T, TOPK, H).sum(axis=1)
    shared_out = np.concatenate([results[c]["ys"] for c in range(NCORES)], axis=0)
    return (expert_out + shared_out).reshape(B, S, H).astype(np.float32)


def kernel(hidden_states, gate_w, w_gate, w_up, w_down, sw_gate, sw_up, sw_down):
    slot_caps, in_maps, meta = prepare(hidden_states, gate_w, w_gate, w_up,
                                       w_down, sw_gate, sw_up, sw_down)
    nc = build_bass(slot_caps)
    global LAST_NC, LAST_RESULTS
    LAST_NC = nc
    try:
        res = run_bass_kernel_spmd(nc, in_maps, core_ids=list(range(NCORES)))
    except ModuleNotFoundError:
        # BASS_TRACE was requested but this axon build lacks the NTFF
        # profile hook module; rerun without tracing.
        os.environ["BASS_NEVER_TRACE"] = "1"
        res = run_bass_kernel_spmd(nc, in_maps, core_ids=list(range(NCORES)))
    LAST_RESULTS = res
    if res.exec_time_ns is not None:
        print(f"HW exec time: {res.exec_time_ns} ns")
    return combine(res.results, meta)



# revision 5
# speedup vs baseline: 1.7007x; 1.7007x over previous
"""DeepseekV3 MoE layer on 8 Trainium2 NeuronCores (expert-parallel).

Contract: kernel(**inputs) takes the FULL unsharded inputs and returns the
FULL output [4, 2048, 2048] f32.

Strategy (v2 — fp8 DoubleRow expert path):
  - Routing (sigmoid gate + group-limited top-6) computed on host in numpy.
  - Expert parallelism: 32 experts -> 8 cores x 4 slots, assigned by sorted
    token count so every core runs an identical static program.
  - Expert MLP in fp8e4 with perf_mode=DoubleRow (2 contraction rows per PE
    cell).  Precision: x is split hi+lo (lo = e4m3 residual, unscaled) and
    both passes accumulate in PSUM against the same fp8 weights, removing
    the x-quantization error for ~zero extra non-PE work.  Weights are
    pre-scaled (wg/wu x16, wd x32) so fp8 stays in the normal range; the
    1/512 dequant folds into the routing-weight multiply.
  - A = silu(g)*u is quantized to fp8 on the DVE (A holds 16*A_true, max
    ~204 < 240 for this dataset).  11 m-tiles -> 5 DoubleRow pairs + 1
    plain fp8 matmul (no padding).
  - Shared MLP in bf16 (error ~2e-3), data-parallel over tokens.
  - Outputs in bf16; host combine upcasts to f32.
"""
import sys
import os

sys.path.insert(0, "/opt/trn_rl_repo")

import numpy as np
import ml_dtypes

import concourse.bacc as bacc_mod
import concourse.mybir as mybir
import concourse.tile as tile
from concourse.bass_utils import run_bass_kernel_spmd

F32 = mybir.dt.float32
F8 = mybir.dt.float8e4
BF16 = mybir.dt.bfloat16
E4 = ml_dtypes.float8_e4m3
BF = ml_dtypes.bfloat16
DR = mybir.MatmulPerfMode.DoubleRow
P = 128

# Problem constants (hardcoded per contract)
B, S, H = 4, 2048, 2048
T = B * S                      # 8192 tokens
E = 32                         # experts
TOPK = 6
N_GROUPS = 4
N_LIMITED = 2
MI = 1408                      # expert intermediate
SH = 2816                      # shared intermediate
NH = H // P                    # 16 h-tiles
NDH = H // (2 * P)             # 8 h double-tiles
NM = MI // P                   # 11 m-tiles (expert)
NMD = NM // 2                  # 5 full DoubleRow pairs (+1 plain m-tile)
NMS = SH // P                  # 22 m-tiles (shared)
NCORES = 8
NSLOTS = 4
HC = 512                       # stage-2 output column chunk
NHC = H // HC                  # 4
TOK_SH = T // NCORES           # 1024 shared-MLP tokens per core
WS = 16.0                      # wg/wu fp8 pre-scale
WDS = 32.0                     # wd fp8 pre-scale


def _round_up(x, m):
    return ((x + m - 1) // m) * m


def _pieces_of(ck):
    out = []
    off = 0
    while off < ck:
        pl = min(512, ck - off)
        out.append((off, pl))
        off += pl
    return out


def build_bass(slot_caps):
    ncap = sum(slot_caps)
    nt = ncap // P
    nc = bacc_mod.Bacc(trn_type="TRN2")

    xhi = nc.dram_tensor("xhi", [NDH, P, 2, ncap], F8, kind="ExternalInput")
    xlo = nc.dram_tensor("xlo", [NDH, P, 2, ncap], F8, kind="ExternalInput")
    rw = nc.dram_tensor("rw", [P, nt], F32, kind="ExternalInput")
    wg = nc.dram_tensor("wg", [NSLOTS, NM, P, H], F8, kind="ExternalInput")
    wu = nc.dram_tensor("wu", [NSLOTS, NM, P, H], F8, kind="ExternalInput")
    wd = nc.dram_tensor("wd", [NSLOTS, NHC, P, NM * HC], F8, kind="ExternalInput")
    xts = nc.dram_tensor("xts", [H, TOK_SH], BF16, kind="ExternalInput")
    swg = nc.dram_tensor("swg", [NMS, P, H], BF16, kind="ExternalInput")
    swu = nc.dram_tensor("swu", [NMS, P, H], BF16, kind="ExternalInput")
    swd = nc.dram_tensor("swd", [NHC, P, NMS * HC], BF16, kind="ExternalInput")
    y = nc.dram_tensor("y", [ncap, H], BF16, kind="ExternalOutput")
    ys = nc.dram_tensor("ys", [TOK_SH, H], BF16, kind="ExternalOutput")

    ACT = mybir.ActivationFunctionType
    with tile.TileContext(nc) as tc:
        with tc.tile_pool(name="rwp", bufs=1) as rwp:
            rws = rwp.tile([P, nt], F32)
            nc.sync.dma_start(rws[:], rw[:])

            # ---- expert phase (fp8 DoubleRow) ----
            with tc.tile_pool(name="ex", bufs=1) as exp_, \
                 tc.tile_pool(name="ea", bufs=1) as eap, \
                 tc.tile_pool(name="ew", bufs=2) as ew, \
                 tc.tile_pool(name="ewd", bufs=2) as ewd, \
                 tc.tile_pool(name="eio", bufs=3) as eio, \
                 tc.tile_pool(name="eps", bufs=2, space="PSUM") as eps, \
                 tc.tile_pool(name="eps2", bufs=3, space="PSUM") as eps2:
                soff = 0
                for s in range(NSLOTS):
                    cap = slot_caps[s]
                    pieces = _pieces_of(cap)
                    xhi_t = exp_.tile([P, NDH, 2, cap], F8, tag="xhi")
                    xlo_t = exp_.tile([P, NDH, 2, cap], F8, tag="xlo")
                    for dh in range(NDH):
                        nc.sync.dma_start(xhi_t[:, dh],
                                          xhi[dh][:, :, soff:soff + cap])
                        nc.sync.dma_start(xlo_t[:, dh],
                                          xlo[dh][:, :, soff:soff + cap])
                    A = eap.tile([P, NM, cap], F8, tag="A")
                    # stage 1: G = X@Wg, U = X@Wu, A = silu(G/WS)*U
                    for mt in range(NM):
                        wgt = ew.tile([P, NDH, 2, P], F8, tag="wg")
                        wut = ew.tile([P, NDH, 2, P], F8, tag="wu")
                        nc.sync.dma_start(
                            wgt[:], wg[s, mt].rearrange(
                                "p (dh i x) -> p dh i x", i=2, x=P))
                        nc.sync.dma_start(
                            wut[:], wu[s, mt].rearrange(
                                "p (dh i x) -> p dh i x", i=2, x=P))
                        for (po, pl) in pieces:
                            g = eps.tile([P, pl], F32, tag="g")
                            u = eps.tile([P, pl], F32, tag="u")
                            for dst, wt in ((g, wgt), (u, wut)):
                                k = 0
                                for dh in range(NDH):
                                    for src in (xhi_t, xlo_t):
                                        nc.tensor.matmul(
                                            dst[:], wt[:, dh],
                                            src[:, dh, :, po:po + pl],
                                            start=(k == 0),
                                            stop=(k == 2 * NDH - 1),
                                            perf_mode=DR)
                                        k += 1
                            sg = eio.tile([P, pl], F32, tag="sg")
                            nc.scalar.activation(out=sg[:], in_=g[:],
                                                 func=ACT.Silu, scale=1.0 / WS)
                            nc.vector.tensor_mul(out=A[:, mt, po:po + pl],
                                                 in0=sg[:], in1=u[:])
                    # stage 2: Y = A @ Wd (5 DR pairs + 1 plain fp8 mm)
                    for hc in range(NHC):
                        wdt = ewd.tile([P, NM, HC], F8, tag="wd")
                        nc.sync.dma_start(
                            wdt[:], wd[s, hc].rearrange(
                                "p (mt x) -> p mt x", x=HC))
                        for t in range(cap // P):
                            yp = eps2.tile([P, HC], F32, tag="y")
                            for dm in range(NMD):
                                nc.tensor.matmul(
                                    yp[:],
                                    A[:, 2 * dm:2 * dm + 2, t * P:(t + 1) * P],
                                    wdt[:, 2 * dm:2 * dm + 2],
                                    start=(dm == 0), stop=False,
                                    perf_mode=DR)
                            nc.tensor.matmul(
                                yp[:], A[:, NM - 1, t * P:(t + 1) * P],
                                wdt[:, NM - 1], start=False, stop=True)
                            ysb = eio.tile([P, HC], BF16, tag="ysb")
                            ti = soff // P + t
                            nc.scalar.mul(ysb[:], yp[:], rws[:, ti:ti + 1])
                            nc.sync.dma_start(
                                y[soff + t * P: soff + (t + 1) * P,
                                  hc * HC:(hc + 1) * HC], ysb[:])
                    soff += cap

            # ---- shared-expert phase (bf16, 1024 tokens) ----
            with tc.tile_pool(name="sres", bufs=1) as sres, \
                 tc.tile_pool(name="sio", bufs=2) as sio, \
                 tc.tile_pool(name="sps", bufs=2, space="PSUM") as sps:
                A = sres.tile([P, NMS, TOK_SH], BF16)
                with tc.tile_pool(name="sx", bufs=1) as sxp, \
                     tc.tile_pool(name="s1w", bufs=2) as s1w:
                    xtr = sxp.tile([P, NH, TOK_SH], BF16)
                    for h in range(NH):
                        nc.sync.dma_start(xtr[:, h],
                                          xts[h * P:(h + 1) * P, :])
                    for m in range(NMS):
                        wgr = s1w.tile([P, NH, P], BF16, tag="wgr")
                        wur = s1w.tile([P, NH, P], BF16, tag="wur")
                        nc.sync.dma_start(
                            wgr[:], swg[m].rearrange("p (ko x) -> p ko x", x=P))
                        nc.sync.dma_start(
                            wur[:], swu[m].rearrange("p (ko x) -> p ko x", x=P))
                        for (po, pl) in _pieces_of(TOK_SH):
                            g = sps.tile([P, pl], F32, tag="g")
                            u = sps.tile([P, pl], F32, tag="u")
                            for h in range(NH):
                                nc.tensor.matmul(g[:], wgr[:, h],
                                                 xtr[:, h, po:po + pl],
                                                 start=(h == 0), stop=(h == NH - 1))
                            for h in range(NH):
                                nc.tensor.matmul(u[:], wur[:, h],
                                                 xtr[:, h, po:po + pl],
                                                 start=(h == 0), stop=(h == NH - 1))
                            sg = sio.tile([P, pl], F32, tag="sg")
                            nc.scalar.activation(out=sg[:], in_=g[:], func=ACT.Silu)
                            nc.vector.tensor_mul(out=A[:, m, po:po + pl],
                                                 in0=sg[:], in1=u[:])
                with tc.tile_pool(name="s2w", bufs=2) as s2w:
                    for hc in range(NHC):
                        wdr = s2w.tile([P, NMS, HC], BF16, tag="wdr")
                        nc.sync.dma_start(
                            wdr[:], swd[hc].rearrange("p (mt x) -> p mt x", x=HC))
                        for t in range(TOK_SH // P):
                            yp = sps.tile([P, HC], F32, tag="y")
                            for m in range(NMS):
                                nc.tensor.matmul(yp[:], A[:, m, t * P:(t + 1) * P],
                                                 wdr[:, m],
                                                 start=(m == 0), stop=(m == NMS - 1))
                            ysb = sio.tile([P, HC], BF16, tag="ysb")
                            nc.scalar.copy(ysb[:], yp[:])
                            nc.sync.dma_start(
                                ys[t * P:(t + 1) * P, hc * HC:(hc + 1) * HC], ysb[:])
    nc.finalize()
    return nc


def _route(x, gate_w):
    """Replicate the reference routing in numpy fp32."""
    logits = x @ gate_w                                   # [T, E]
    scores = 1.0 / (1.0 + np.exp(-logits))
    sg = scores.reshape(T, N_GROUPS, E // N_GROUPS)
    group_scores = sg.max(axis=-1)
    top_groups = np.argsort(-group_scores, axis=1, kind="stable")[:, :N_LIMITED]
    mask = np.ones((T, N_GROUPS), dtype=bool)
    mask[np.arange(T)[:, None], top_groups] = False
    sgm = np.where(mask[:, :, None], -np.inf, sg).reshape(T, E)
    sel = np.argsort(-sgm, axis=1, kind="stable")[:, :TOPK]     # [T, K]
    w = np.take_along_axis(scores, sel, axis=1)
    w = w / w.sum(axis=1, keepdims=True)
    return sel.astype(np.int64), w.astype(np.float32)


def _q8(a):
    return np.clip(a, -240.0, 240.0).astype(E4)


def prepare(hidden_states, gate_w, w_gate, w_up, w_down, sw_gate, sw_up, sw_down):
    """Host-side routing + quantization + sharding."""
    x = np.ascontiguousarray(np.asarray(hidden_states, dtype=np.float32).reshape(T, H))
    gate_w = np.asarray(gate_w, dtype=np.float32)
    w_gate = np.asarray(w_gate, dtype=np.float32)
    w_up = np.asarray(w_up, dtype=np.float32)
    w_down = np.asarray(w_down, dtype=np.float32)
    sw_gate = np.asarray(sw_gate, dtype=np.float32)
    sw_up = np.asarray(sw_up, dtype=np.float32)
    sw_down = np.asarray(sw_down, dtype=np.float32)

    # ---- 1. routing ----
    sel, wts = _route(x, gate_w)
    sel_flat = sel.ravel()                       # pair index -> expert
    counts = np.bincount(sel_flat, minlength=E)

    # ---- 2. expert -> (core, slot) assignment ----
    order = np.argsort(-counts, kind="stable")   # experts by count desc
    slot_caps = []
    assign = np.empty((NCORES, NSLOTS), dtype=np.int64)
    for s in range(NSLOTS):
        grp = order[s * NCORES:(s + 1) * NCORES]
        assign[:, s] = grp
        slot_caps.append(max(P, _round_up(int(counts[grp].max()), P)))
    ncap = sum(slot_caps)
    soffs = np.cumsum([0] + slot_caps)[:-1]

    rows_of = [np.flatnonzero(sel_flat == e) for e in range(E)]

    # ---- 3. global fp8 quantization of x (hi + residual lo) ----
    xhi_q = _q8(x)                               # [T, H] fp8
    xlo_q = _q8(x - xhi_q.astype(np.float32))

    # ---- 4. shared tensors (identical on every core) ----
    swg_t = np.ascontiguousarray(
        sw_gate.reshape(NH, P, NMS, P).transpose(2, 1, 0, 3)
        .reshape(NMS, P, H)).astype(BF)
    swu_t = np.ascontiguousarray(
        sw_up.reshape(NH, P, NMS, P).transpose(2, 1, 0, 3)
        .reshape(NMS, P, H)).astype(BF)
    swd_t = np.ascontiguousarray(
        sw_down.reshape(NMS, P, NHC, HC).transpose(2, 1, 0, 3)
        .reshape(NHC, P, NMS * HC)).astype(BF)

    in_maps = []
    for c in range(NCORES):
        el = assign[c]                            # 4 expert ids
        xh_c = np.zeros((ncap, H), dtype=E4)
        xl_c = np.zeros((ncap, H), dtype=E4)
        rw_c = np.zeros(ncap, dtype=np.float32)
        for s in range(NSLOTS):
            e = el[s]
            r = rows_of[e]
            n = len(r)
            xh_c[soffs[s]:soffs[s] + n] = xhi_q[r // TOPK]
            xl_c[soffs[s]:soffs[s] + n] = xlo_q[r // TOPK]
            rw_c[soffs[s]:soffs[s] + n] = wts[r // TOPK, r % TOPK] / (WS * WDS)
        # [ncap, H] -> [NDH, P, 2, ncap]
        xh_c = np.ascontiguousarray(
            xh_c.reshape(ncap, NDH, 2, P).transpose(1, 3, 2, 0))
        xl_c = np.ascontiguousarray(
            xl_c.reshape(ncap, NDH, 2, P).transpose(1, 3, 2, 0))
        rw_t = np.ascontiguousarray(rw_c.reshape(ncap // P, P).T)   # [128, nt]

        # wg/wu: [H, MI] -> [NM, P(kpart), H] with k = (dh, i, p) order
        wg_c = np.ascontiguousarray(
            _q8(w_gate[el] * WS).reshape(NSLOTS, NDH, 2, P, NM, P)
            .transpose(0, 4, 3, 1, 2, 5).reshape(NSLOTS, NM, P, H))
        wu_c = np.ascontiguousarray(
            _q8(w_up[el] * WS).reshape(NSLOTS, NDH, 2, P, NM, P)
            .transpose(0, 4, 3, 1, 2, 5).reshape(NSLOTS, NM, P, H))
        # wd: [MI, H] -> [NHC, P, NM*HC]
        wd_c = np.ascontiguousarray(
            _q8(w_down[el] * WDS).reshape(NSLOTS, NM, P, NHC, HC)
            .transpose(0, 3, 2, 1, 4).reshape(NSLOTS, NHC, P, NM * HC))

        xts_c = np.ascontiguousarray(
            x[c * TOK_SH:(c + 1) * TOK_SH].T).astype(BF)  # [H, 1024]

        in_maps.append({
            "xhi": xh_c, "xlo": xl_c, "rw": rw_t,
            "wg": wg_c, "wu": wu_c, "wd": wd_c,
            "xts": xts_c, "swg": swg_t, "swu": swu_t, "swd": swd_t,
        })

    meta = {"rows_of": rows_of, "assign": assign, "soffs": soffs}
    return slot_caps, in_maps, meta


def combine(results, meta):
    """Host-side unshard: scatter expert outputs back + add shared."""
    rows_of, assign, soffs = meta["rows_of"], meta["assign"], meta["soffs"]
    d_pairs = np.empty((T * TOPK, H), dtype=np.float32)
    for c in range(NCORES):
        y_c = results[c]["y"].astype(np.float32)
        for s in range(NSLOTS):
            r = rows_of[assign[c, s]]
            d_pairs[r] = y_c[soffs[s]:soffs[s] + len(r)]
    expert_out = d_pairs.reshape(T, TOPK, H).sum(axis=1)
    shared_out = np.concatenate(
        [results[c]["ys"].astype(np.float32) for c in range(NCORES)], axis=0)
    return (expert_out + shared_out).reshape(B, S, H).astype(np.float32)


def kernel(hidden_states, gate_w, w_gate, w_up, w_down, sw_gate, sw_up, sw_down):
    slot_caps, in_maps, meta = prepare(hidden_states, gate_w, w_gate, w_up,
                                       w_down, sw_gate, sw_up, sw_down)
    nc = build_bass(slot_caps)
    global LAST_NC, LAST_RESULTS
    LAST_NC = nc
    try:
        res = run_bass_kernel_spmd(nc, in_maps, core_ids=list(range(NCORES)))
    except ModuleNotFoundError:
        # BASS_TRACE was requested but this axon build lacks the NTFF
        # profile hook module; rerun without tracing.
        os.environ["BASS_NEVER_TRACE"] = "1"
        res = run_bass_kernel_spmd(nc, in_maps, core_ids=list(range(NCORES)))
    LAST_RESULTS = res
    if res.exec_time_ns is not None:
        print(f"HW exec time: {res.exec_time_ns} ns")
    return combine(res.results, meta)


# revision 6
# speedup vs baseline: 1.9095x; 1.1228x over previous
"""DeepseekV3 MoE layer on 8 Trainium2 NeuronCores (expert-parallel).

Contract: kernel(**inputs) takes the FULL unsharded inputs and returns the
FULL output [4, 2048, 2048] f32.

Strategy (v3 — fp8 DoubleRow everywhere):
  - Routing (sigmoid gate + group-limited top-6) computed on host in numpy.
  - Expert parallelism: 32 experts -> 8 cores x 4 slots, assigned by sorted
    token count so every core runs an identical static program.
  - Expert MLP in fp8e4 with perf_mode=DoubleRow.  Precision: x is split
    hi+lo (lo = e4m3 residual, unscaled) and both passes accumulate in PSUM
    against the same fp8 weights, removing the x-quantization error for
    ~zero extra non-PE work.  Weights pre-scaled (wg/wu x16, wd x32); the
    1/512 dequant folds into the routing-weight multiply.  A = silu(g)*u is
    quantized to fp8 on the DVE; 11 m-tiles padded to 12 (A[:,11] memset,
    wd zero-padded) so stage 2 is 6 clean DoubleRow pairs.
  - Shared MLP also fp8 DoubleRow, 3-pass error-feedback: stage 1 g/u =
    xhi@w8 + xlo@w8 + xhi@wres (wres = fp8 residual of the weight quant);
    stage 2 y = Ahi@wd8 + Alo@wd8 + Ahi@wdres with the A hi/lo split done
    on-device (DVE).  More accurate than bf16 at 0.75x the PE cost.
  - Outputs in bf16; host combine upcasts to f32.
"""
import sys
import os

sys.path.insert(0, "/opt/trn_rl_repo")

import numpy as np
import ml_dtypes

import concourse.bacc as bacc_mod
import concourse.mybir as mybir
import concourse.tile as tile
from concourse.bass_utils import run_bass_kernel_spmd

F32 = mybir.dt.float32
F8 = mybir.dt.float8e4
BF16 = mybir.dt.bfloat16
E4 = ml_dtypes.float8_e4m3
BF = ml_dtypes.bfloat16
DR = mybir.MatmulPerfMode.DoubleRow
P = 128

# Problem constants (hardcoded per contract)
B, S, H = 4, 2048, 2048
T = B * S                      # 8192 tokens
E = 32                         # experts
TOPK = 6
N_GROUPS = 4
N_LIMITED = 2
MI = 1408                      # expert intermediate
SH = 2816                      # shared intermediate
NH = H // P                    # 16 h-tiles
NDH = H // (2 * P)             # 8 h double-tiles
NM = MI // P                   # 11 m-tiles (expert)
NM2 = NM + 1                   # padded to 6 DoubleRow pairs
NMS = SH // P                  # 22 m-tiles (shared) -> 11 DR pairs
NCORES = 8
NSLOTS = 4
HC = 512                       # stage-2 output column chunk
NHC = H // HC                  # 4
TOK_SH = T // NCORES           # 1024 shared-MLP tokens per core
WS = 16.0                      # stage-1 weight fp8 pre-scale
WDS = 32.0                     # stage-2 weight fp8 pre-scale


def _round_up(x, m):
    return ((x + m - 1) // m) * m


def _pieces_of(ck):
    out = []
    off = 0
    while off < ck:
        pl = min(512, ck - off)
        out.append((off, pl))
        off += pl
    return out


def build_bass(slot_caps):
    ncap = sum(slot_caps)
    nt = ncap // P
    nc = bacc_mod.Bacc(trn_type="TRN2")

    xhi = nc.dram_tensor("xhi", [NDH, P, 2, ncap], F8, kind="ExternalInput")
    xlo = nc.dram_tensor("xlo", [NDH, P, 2, ncap], F8, kind="ExternalInput")
    rw = nc.dram_tensor("rw", [P, nt], F32, kind="ExternalInput")
    wg = nc.dram_tensor("wg", [NSLOTS, NM, P, H], F8, kind="ExternalInput")
    wu = nc.dram_tensor("wu", [NSLOTS, NM, P, H], F8, kind="ExternalInput")
    wd = nc.dram_tensor("wd", [NSLOTS, NHC, P, NM2 * HC], F8, kind="ExternalInput")
    xshi = nc.dram_tensor("xshi", [NDH, P, 2, TOK_SH], F8, kind="ExternalInput")
    xslo = nc.dram_tensor("xslo", [NDH, P, 2, TOK_SH], F8, kind="ExternalInput")
    swg8 = nc.dram_tensor("swg8", [NMS, P, H], F8, kind="ExternalInput")
    swgr = nc.dram_tensor("swgr", [NMS, P, H], F8, kind="ExternalInput")
    swu8 = nc.dram_tensor("swu8", [NMS, P, H], F8, kind="ExternalInput")
    swur = nc.dram_tensor("swur", [NMS, P, H], F8, kind="ExternalInput")
    swd8 = nc.dram_tensor("swd8", [NHC, P, NMS * HC], F8, kind="ExternalInput")
    swdr = nc.dram_tensor("swdr", [NHC, P, NMS * HC], F8, kind="ExternalInput")
    y = nc.dram_tensor("y", [ncap, H], BF16, kind="ExternalOutput")
    ys = nc.dram_tensor("ys", [TOK_SH, H], BF16, kind="ExternalOutput")

    ACT = mybir.ActivationFunctionType
    with tile.TileContext(nc) as tc:
        with tc.tile_pool(name="rwp", bufs=1) as rwp:
            rws = rwp.tile([P, nt], F32)
            nc.sync.dma_start(rws[:], rw[:])

            # ---- expert phase (fp8 DoubleRow, x hi+lo 2-pass) ----
            with tc.tile_pool(name="ex", bufs=1) as exp_, \
                 tc.tile_pool(name="ea", bufs=2) as eap, \
                 tc.tile_pool(name="ew", bufs=2) as ew, \
                 tc.tile_pool(name="ewd", bufs=2) as ewd, \
                 tc.tile_pool(name="eio", bufs=3) as eio, \
                 tc.tile_pool(name="eps", bufs=2, space="PSUM") as eps, \
                 tc.tile_pool(name="eps2", bufs=3, space="PSUM") as eps2:
                soff = 0
                for s in range(NSLOTS):
                    cap = slot_caps[s]
                    pieces = _pieces_of(cap)
                    xhi_t = exp_.tile([P, NDH, 2, cap], F8, tag="xhi")
                    xlo_t = exp_.tile([P, NDH, 2, cap], F8, tag="xlo")
                    for dh in range(NDH):
                        nc.sync.dma_start(xhi_t[:, dh],
                                          xhi[dh][:, :, soff:soff + cap])
                        nc.sync.dma_start(xlo_t[:, dh],
                                          xlo[dh][:, :, soff:soff + cap])
                    A = eap.tile([P, NM2, cap], F8, tag="A")
                    nc.gpsimd.memset(A[:, NM2 - 1], 0.0)
                    # stage 1: G = X@Wg, U = X@Wu, A = silu(G/WS)*U
                    for mt in range(NM):
                        wgt = ew.tile([P, NDH, 2, P], F8, tag="wg")
                        wut = ew.tile([P, NDH, 2, P], F8, tag="wu")
                        nc.sync.dma_start(
                            wgt[:], wg[s, mt].rearrange(
                                "p (dh i x) -> p dh i x", i=2, x=P))
                        nc.sync.dma_start(
                            wut[:], wu[s, mt].rearrange(
                                "p (dh i x) -> p dh i x", i=2, x=P))
                        for (po, pl) in pieces:
                            g = eps.tile([P, pl], F32, tag="g")
                            u = eps.tile([P, pl], F32, tag="u")
                            for dst, wt in ((g, wgt), (u, wut)):
                                k = 0
                                for dh in range(NDH):
                                    for src in (xhi_t, xlo_t):
                                        nc.tensor.matmul(
                                            dst[:], wt[:, dh],
                                            src[:, dh, :, po:po + pl],
                                            start=(k == 0),
                                            stop=(k == 2 * NDH - 1),
                                            perf_mode=DR)
                                        k += 1
                            sg = eio.tile([P, pl], F32, tag="sg")
                            nc.scalar.activation(out=sg[:], in_=g[:],
                                                 func=ACT.Silu, scale=1.0 / WS)
                            nc.vector.tensor_mul(out=A[:, mt, po:po + pl],
                                                 in0=sg[:], in1=u[:])
                    # stage 2: Y = A @ Wd (6 DoubleRow pairs)
                    for hc in range(NHC):
                        wdt = ewd.tile([P, NM2, HC], F8, tag="wd")
                        nc.sync.dma_start(
                            wdt[:], wd[s, hc].rearrange(
                                "p (mt x) -> p mt x", x=HC))
                        for t in range(cap // P):
                            yp = eps2.tile([P, HC], F32, tag="y")
                            for dm in range(NM2 // 2):
                                nc.tensor.matmul(
                                    yp[:],
                                    A[:, 2 * dm:2 * dm + 2, t * P:(t + 1) * P],
                                    wdt[:, 2 * dm:2 * dm + 2],
                                    start=(dm == 0), stop=(dm == NM2 // 2 - 1),
                                    perf_mode=DR)
                            ysb = eio.tile([P, HC], BF16, tag="ysb")
                            ti = soff // P + t
                            nc.scalar.mul(ysb[:], yp[:], rws[:, ti:ti + 1])
                            nc.sync.dma_start(
                                y[soff + t * P: soff + (t + 1) * P,
                                  hc * HC:(hc + 1) * HC], ysb[:])
                    soff += cap

            # ---- shared-expert phase (fp8 DoubleRow, 3-pass) ----
            with tc.tile_pool(name="sa", bufs=1) as sap, \
                 tc.tile_pool(name="sio", bufs=3) as sio, \
                 tc.tile_pool(name="sps", bufs=2, space="PSUM") as sps, \
                 tc.tile_pool(name="sps2", bufs=3, space="PSUM") as sps2:
                Ahi = sap.tile([P, NMS, TOK_SH], F8)
                Alo = sap.tile([P, NMS, TOK_SH], F8)
                with tc.tile_pool(name="sx", bufs=1) as sxp, \
                     tc.tile_pool(name="s1w", bufs=2) as s1w:
                    xshi_t = sxp.tile([P, NDH, 2, TOK_SH], F8, tag="xh")
                    xslo_t = sxp.tile([P, NDH, 2, TOK_SH], F8, tag="xl")
                    for dh in range(NDH):
                        nc.sync.dma_start(xshi_t[:, dh], xshi[dh])
                        nc.sync.dma_start(xslo_t[:, dh], xslo[dh])
                    for mt in range(NMS):
                        wgt = s1w.tile([P, NDH, 2, P], F8, tag="wg")
                        wgr = s1w.tile([P, NDH, 2, P], F8, tag="wgr")
                        wut = s1w.tile([P, NDH, 2, P], F8, tag="wu")
                        wur = s1w.tile([P, NDH, 2, P], F8, tag="wur")
                        for t_, d_ in ((wgt, swg8), (wgr, swgr),
                                       (wut, swu8), (wur, swur)):
                            nc.sync.dma_start(
                                t_[:], d_[mt].rearrange(
                                    "p (dh i x) -> p dh i x", i=2, x=P))
                        for (po, pl) in _pieces_of(TOK_SH):
                            g = sps.tile([P, pl], F32, tag="g")
                            u = sps.tile([P, pl], F32, tag="u")
                            for dst, whi, wre in ((g, wgt, wgr), (u, wut, wur)):
                                k = 0
                                for dh in range(NDH):
                                    for src, wt in ((xshi_t, whi),
                                                    (xslo_t, whi),
                                                    (xshi_t, wre)):
                                        nc.tensor.matmul(
                                            dst[:], wt[:, dh],
                                            src[:, dh, :, po:po + pl],
                                            start=(k == 0),
                                            stop=(k == 3 * NDH - 1),
                                            perf_mode=DR)
                                        k += 1
                            sg = sio.tile([P, pl], F32, tag="sg")
                            nc.scalar.activation(out=sg[:], in_=g[:],
                                                 func=ACT.Silu, scale=1.0 / WS)
                            tfull = sio.tile([P, pl], F32, tag="t")
                            nc.vector.tensor_mul(out=tfull[:], in0=sg[:], in1=u[:])
                            nc.scalar.copy(Ahi[:, mt, po:po + pl], tfull[:])
                            nc.vector.tensor_sub(out=Alo[:, mt, po:po + pl],
                                                 in0=tfull[:],
                                                 in1=Ahi[:, mt, po:po + pl])
                with tc.tile_pool(name="s2w", bufs=2) as s2w:
                    for hc in range(NHC):
                        wdt = s2w.tile([P, NMS, HC], F8, tag="wd8")
                        wdr = s2w.tile([P, NMS, HC], F8, tag="wdr")
                        nc.sync.dma_start(
                            wdt[:], swd8[hc].rearrange("p (mt x) -> p mt x", x=HC))
                        nc.sync.dma_start(
                            wdr[:], swdr[hc].rearrange("p (mt x) -> p mt x", x=HC))
                        for t in range(TOK_SH // P):
                            yp = sps2.tile([P, HC], F32, tag="y")
                            k = 0
                            for At, wt in ((Ahi, wdt), (Alo, wdt), (Ahi, wdr)):
                                for dm in range(NMS // 2):
                                    nc.tensor.matmul(
                                        yp[:],
                                        At[:, 2 * dm:2 * dm + 2, t * P:(t + 1) * P],
                                        wt[:, 2 * dm:2 * dm + 2],
                                        start=(k == 0),
                                        stop=(k == 3 * (NMS // 2) - 1),
                                        perf_mode=DR)
                                    k += 1
                            ysb = sio.tile([P, HC], BF16, tag="ysb")
                            nc.scalar.mul(ysb[:], yp[:], 1.0 / (WS * WDS))
                            nc.sync.dma_start(
                                ys[t * P:(t + 1) * P, hc * HC:(hc + 1) * HC], ysb[:])
    nc.finalize()
    return nc


def _route(x, gate_w):
    """Replicate the reference routing in numpy fp32."""
    logits = x @ gate_w                                   # [T, E]
    scores = 1.0 / (1.0 + np.exp(-logits))
    sg = scores.reshape(T, N_GROUPS, E // N_GROUPS)
    group_scores = sg.max(axis=-1)
    top_groups = np.argsort(-group_scores, axis=1, kind="stable")[:, :N_LIMITED]
    mask = np.ones((T, N_GROUPS), dtype=bool)
    mask[np.arange(T)[:, None], top_groups] = False
    sgm = np.where(mask[:, :, None], -np.inf, sg).reshape(T, E)
    sel = np.argsort(-sgm, axis=1, kind="stable")[:, :TOPK]     # [T, K]
    w = np.take_along_axis(scores, sel, axis=1)
    w = w / w.sum(axis=1, keepdims=True)
    return sel.astype(np.int64), w.astype(np.float32)


def _q8(a):
    return np.clip(a, -240.0, 240.0).astype(E4)


def _pack_pairs_w(wq, n_mt):
    """[E?, H, M] fp8 (k-major) -> [..., n_mt, P, H] with k order (dh, i, p)."""
    lead = wq.shape[:-2]
    return np.ascontiguousarray(
        wq.reshape(*lead, NDH, 2, P, n_mt, P)
        .transpose(*range(len(lead)), len(lead) + 3, len(lead) + 2,
                   len(lead), len(lead) + 1, len(lead) + 4)
        .reshape(*lead, n_mt, P, H))


def _pack_x_pairs(xq):
    """[N, H] fp8 -> [NDH, P, 2, N]"""
    n = xq.shape[0]
    return np.ascontiguousarray(xq.reshape(n, NDH, 2, P).transpose(1, 3, 2, 0))


def prepare(hidden_states, gate_w, w_gate, w_up, w_down, sw_gate, sw_up, sw_down):
    """Host-side routing + quantization + sharding."""
    x = np.ascontiguousarray(np.asarray(hidden_states, dtype=np.float32).reshape(T, H))
    gate_w = np.asarray(gate_w, dtype=np.float32)
    w_gate = np.asarray(w_gate, dtype=np.float32)
    w_up = np.asarray(w_up, dtype=np.float32)
    w_down = np.asarray(w_down, dtype=np.float32)
    sw_gate = np.asarray(sw_gate, dtype=np.float32)
    sw_up = np.asarray(sw_up, dtype=np.float32)
    sw_down = np.asarray(sw_down, dtype=np.float32)

    # ---- 1. routing ----
    sel, wts = _route(x, gate_w)
    sel_flat = sel.ravel()                       # pair index -> expert
    counts = np.bincount(sel_flat, minlength=E)

    # ---- 2. expert -> (core, slot) assignment ----
    order = np.argsort(-counts, kind="stable")   # experts by count desc
    slot_caps = []
    assign = np.empty((NCORES, NSLOTS), dtype=np.int64)
    for s in range(NSLOTS):
        grp = order[s * NCORES:(s + 1) * NCORES]
        assign[:, s] = grp
        slot_caps.append(max(P, _round_up(int(counts[grp].max()), P)))
    ncap = sum(slot_caps)
    soffs = np.cumsum([0] + slot_caps)[:-1]

    rows_of = [np.flatnonzero(sel_flat == e) for e in range(E)]

    # ---- 3. global fp8 quantization of x (hi + residual lo) ----
    xhi_q = _q8(x)                               # [T, H] fp8
    xlo_q = _q8(x - xhi_q.astype(np.float32))

    # ---- 4. shared tensors (identical on every core) ----
    sg_s = sw_gate * WS
    sg_hi = _q8(sg_s)
    sg_re = _q8(sg_s - sg_hi.astype(np.float32))
    su_s = sw_up * WS
    su_hi = _q8(su_s)
    su_re = _q8(su_s - su_hi.astype(np.float32))
    swg8_t = _pack_pairs_w(sg_hi, NMS)
    swgr_t = _pack_pairs_w(sg_re, NMS)
    swu8_t = _pack_pairs_w(su_hi, NMS)
    swur_t = _pack_pairs_w(su_re, NMS)
    sd_s = sw_down * WDS
    sd_hi = _q8(sd_s)
    sd_re = _q8(sd_s - sd_hi.astype(np.float32))
    swd8_t = np.ascontiguousarray(
        sd_hi.reshape(NMS, P, NHC, HC).transpose(2, 1, 0, 3)
        .reshape(NHC, P, NMS * HC))
    swdr_t = np.ascontiguousarray(
        sd_re.reshape(NMS, P, NHC, HC).transpose(2, 1, 0, 3)
        .reshape(NHC, P, NMS * HC))

    in_maps = []
    for c in range(NCORES):
        el = assign[c]                            # 4 expert ids
        xh_c = np.zeros((ncap, H), dtype=E4)
        xl_c = np.zeros((ncap, H), dtype=E4)
        rw_c = np.zeros(ncap, dtype=np.float32)
        for s in range(NSLOTS):
            e = el[s]
            r = rows_of[e]
            n = len(r)
            xh_c[soffs[s]:soffs[s] + n] = xhi_q[r // TOPK]
            xl_c[soffs[s]:soffs[s] + n] = xlo_q[r // TOPK]
            rw_c[soffs[s]:soffs[s] + n] = wts[r // TOPK, r % TOPK] / (WS * WDS)
        rw_t = np.ascontiguousarray(rw_c.reshape(ncap // P, P).T)   # [128, nt]

        wg_c = _pack_pairs_w(_q8(w_gate[el] * WS), NM)
        wu_c = _pack_pairs_w(_q8(w_up[el] * WS), NM)
        # wd: [MI, H] -> pad to NM2 m-tiles -> [NHC, P, NM2*HC]
        wdq = np.zeros((NSLOTS, NM2 * P, H), dtype=E4)
        wdq[:, :MI] = _q8(w_down[el] * WDS)
        wd_c = np.ascontiguousarray(
            wdq.reshape(NSLOTS, NM2, P, NHC, HC)
            .transpose(0, 3, 2, 1, 4).reshape(NSLOTS, NHC, P, NM2 * HC))

        in_maps.append({
            "xhi": _pack_x_pairs(xh_c), "xlo": _pack_x_pairs(xl_c), "rw": rw_t,
            "wg": wg_c, "wu": wu_c, "wd": wd_c,
            "xshi": _pack_x_pairs(xhi_q[c * TOK_SH:(c + 1) * TOK_SH]),
            "xslo": _pack_x_pairs(xlo_q[c * TOK_SH:(c + 1) * TOK_SH]),
            "swg8": swg8_t, "swgr": swgr_t, "swu8": swu8_t, "swur": swur_t,
            "swd8": swd8_t, "swdr": swdr_t,
        })

    meta = {"rows_of": rows_of, "assign": assign, "soffs": soffs}
    return slot_caps, in_maps, meta


def combine(results, meta):
    """Host-side unshard: scatter expert outputs back + add shared."""
    rows_of, assign, soffs = meta["rows_of"], meta["assign"], meta["soffs"]
    d_pairs = np.empty((T * TOPK, H), dtype=np.float32)
    for c in range(NCORES):
        y_c = results[c]["y"].astype(np.float32)
        for s in range(NSLOTS):
            r = rows_of[assign[c, s]]
            d_pairs[r] = y_c[soffs[s]:soffs[s] + len(r)]
    expert_out = d_pairs.reshape(T, TOPK, H).sum(axis=1)
    shared_out = np.concatenate(
        [results[c]["ys"].astype(np.float32) for c in range(NCORES)], axis=0)
    return (expert_out + shared_out).reshape(B, S, H).astype(np.float32)


def kernel(hidden_states, gate_w, w_gate, w_up, w_down, sw_gate, sw_up, sw_down):
    slot_caps, in_maps, meta = prepare(hidden_states, gate_w, w_gate, w_up,
                                       w_down, sw_gate, sw_up, sw_down)
    nc = build_bass(slot_caps)
    global LAST_NC, LAST_RESULTS
    LAST_NC = nc
    try:
        res = run_bass_kernel_spmd(nc, in_maps, core_ids=list(range(NCORES)))
    except ModuleNotFoundError:
        # BASS_TRACE was requested but this axon build lacks the NTFF
        # profile hook module; rerun without tracing.
        os.environ["BASS_NEVER_TRACE"] = "1"
        res = run_bass_kernel_spmd(nc, in_maps, core_ids=list(range(NCORES)))
    LAST_RESULTS = res
    if res.exec_time_ns is not None:
        print(f"HW exec time: {res.exec_time_ns} ns")
    return combine(res.results, meta)
